# revision 1
# baseline (speedup 1.0000x reference)
"""Causal multi-head attention for TRN2, 8 NeuronCores — bf16 pipeline.

Sharding: core c -> batch c//2, head-half c%2 (6 heads = 3 pairs).
Per-core: all matmuls bf16 (fp32 PSUM). Per pair p, q-chunk qc (512 cols):
per k-block kc (128 rows): scores S^T[k,q] for both heads (partition bases
0/64 -> concurrent PE row groups); diagonal blocks get -240*upper-tri added
via a matmul (identity stationary, negtri moving) instead of a DVE mask;
one exp instr per kc covers both heads [P, 2, 512-c0] (PSUM->SBUF bf16);
z^T[h,q] (+colsum row from the V ones column) accumulates over kc.
Softmax: reciprocal of colsum (DVE, bf16), broadcast across 64 partitions
with a K=1 matmul, then copy+multiply into the Z stack (bf16).
Out: out^T[m,s] = W_O^T @ Z + folded bias c, bf16 DMA out (host upcasts).
QKV/out projections are emitted as "filler units" interleaved into the
attention loop so the PE queue has no bubbles while ACT streams exps.

Per-rep wall time is DMA-queue-bound, not compute-bound: a single DGE
queue sustains only ~25-30 GB/s, so weights/constants are loaded once
outside the rep loop (resident in SBUF) and the per-rep activation
traffic is spread across three queues (xT split over sync+gpsimd, the
bf16 output over scalar+sync).
"""

from collections import deque

import numpy as np
import ml_dtypes

import concourse.mybir as mybir
import concourse.tile as tile
from concourse import bacc
from concourse.bass_utils import run_bass_kernel_spmd

B, S, DM, NH, DH = 4, 2048, 768, 12, 64
P = 128
HPC = NH // 2            # heads per core: 6
NPAIR = HPC // 2         # head pairs per core: 3
MC = DM // P             # 6 contraction chunks of 128 over d_model
QC = S // 512            # 4 q chunks of 512
KC = S // P              # 16 k chunks of 128
NCORES = 8

F32 = mybir.dt.float32
BF16 = mybir.dt.bfloat16


def build_nc(reps: int = 1, dyn_reps: int | None = None):
    nc = bacc.Bacc(None, target_bir_lowering=False)

    xT = nc.dram_tensor("xT", [DM, S], BF16, kind="ExternalInput")
    wq = nc.dram_tensor("wq", [DM, HPC * DH], BF16, kind="ExternalInput")
    wk = nc.dram_tensor("wk", [DM, HPC * DH], BF16, kind="ExternalInput")
    wv = nc.dram_tensor("wv", [DM, HPC * DH], BF16, kind="ExternalInput")
    wo = nc.dram_tensor("wo", [HPC * DH, DM], BF16, kind="ExternalInput")
    # packed per-partition vectors: b_Q pairs (3), b_K pairs (3), c bias (6)
    bias = nc.dram_tensor("bias", [P, 2 * NPAIR + MC], F32, kind="ExternalInput")
    # cst[:, 0:128] = negtri (-240 where k > q), cst[:, 128:256] = identity
    cst = nc.dram_tensor("cst", [P, 2 * P], BF16, kind="ExternalInput")
    out = nc.dram_tensor("out", [DM, S], BF16, kind="ExternalOutput")

    with tile.TileContext(nc) as tc:
        with (
            tc.tile_pool(name="fx", bufs=1) as fx,
            tc.tile_pool(name="qk", bufs=2) as qkp,
            tc.tile_pool(name="es", bufs=3) as esp,
            tc.tile_pool(name="rcl", bufs=2) as rcp,
            tc.tile_pool(name="ot", bufs=4) as otp,
            tc.tile_pool(name="psS", bufs=2, space="PSUM") as psS,
            tc.tile_pool(name="psZ", bufs=1, space="PSUM") as psZ,
            tc.tile_pool(name="psP", bufs=1, space="PSUM") as psP,
            tc.tile_pool(name="psB", bufs=1, space="PSUM") as psB,
        ):
            # constants + weights: loaded once, resident across reps
            cst_sb = fx.tile([P, 2 * P], BF16, tag="cst")
            nc.gpsimd.dma_start(cst_sb[:], cst[:])
            bias_sb = fx.tile([P, 2 * NPAIR + MC], F32, tag="bias")
            nc.gpsimd.dma_start(bias_sb[:], bias[:])
            negtri = cst_sb[:, 0:P]
            ident = cst_sb[:, P:2 * P]
            bq_sb = bias_sb[:, 0:NPAIR]
            bk_sb = bias_sb[:, NPAIR:2 * NPAIR]
            cb_sb = bias_sb[:, 2 * NPAIR:]
            wq_sb = fx.tile([P, MC, HPC * DH], BF16, tag="wq")
            wk_sb = fx.tile([P, MC, HPC * DH], BF16, tag="wk")
            wv_sb = fx.tile([P, MC, HPC * DH], BF16, tag="wv")
            for w_sb, w_dr in ((wq_sb, wq), (wk_sb, wk), (wv_sb, wv)):
                nc.gpsimd.dma_start(
                    w_sb[:], w_dr.rearrange("(c p) h -> p c h", p=P)
                )
            wo_sb = fx.tile([P, NPAIR, DM], BF16, tag="wo")
            nc.gpsimd.dma_start(wo_sb[:], wo.rearrange("(c p) m -> p c m", p=P))
            ones64 = fx.tile([P, DH], BF16, tag="ones64")
            nc.gpsimd.memset(ones64[:], 1.0)
            V_all = fx.tile([P, KC, HPC, DH + 1], BF16, tag="V")
            nc.gpsimd.memset(V_all[:, :, :, DH], 1.0)
            Z = fx.tile([P, NPAIR, S], BF16, tag="Z")
            xT_sb = fx.tile([P, MC, S], BF16, tag="xT")

            def emit_body():
                # ---- per-rep input DMA: spread across sync + gpsimd queues
                for mi in range(MC):
                    eng = nc.sync if mi % 3 == 0 else nc.gpsimd
                    eng.dma_start(xT_sb[:, mi, :], xT[mi * P:(mi + 1) * P, :])

                # ---- deferred projection "filler" units ----
                qkt: dict[int, tuple] = {}

                def alloc_qk(p_):
                    qt = qkp.tile([P, S], BF16, tag="qt", name="qt")
                    kt = qkp.tile([P, S], BF16, tag="kt", name="kt")
                    qkt[p_] = (qt, kt)

                def qk_unit(p_, ti, qc_):
                    t_sb = qkt[p_][ti]
                    w_sb = (wq_sb, wk_sb)[ti]
                    b_sb = (bq_sb, bk_sb)[ti]
                    pj = psP.tile([P, 512], F32, tag="pj", name="pj")
                    for mi in range(MC):
                        nc.tensor.matmul(
                            pj[:],
                            w_sb[:, mi, 2 * p_ * DH:(2 * p_ + 2) * DH],
                            xT_sb[:, mi, qc_ * 512:(qc_ + 1) * 512],
                            start=(mi == 0),
                            stop=(mi == MC - 1),
                        )
                    nc.vector.tensor_scalar_add(
                        t_sb[:, qc_ * 512:(qc_ + 1) * 512], pj[:], b_sb[:, p_:p_ + 1]
                    )

                def v_unit(sc):
                    pj = psP.tile([P, 512], F32, tag="pj", name="pj")
                    vpj = pj[:, :HPC * DH]
                    for mi in range(MC):
                        nc.tensor.matmul(
                            vpj,
                            xT_sb[:, mi, sc * P:(sc + 1) * P],
                            wv_sb[:, mi, :],
                            start=(mi == 0),
                            stop=(mi == MC - 1),
                        )
                    nc.vector.tensor_copy(
                        V_all[:, sc, :, 0:DH],
                        vpj.rearrange("p (n h) -> p n h", n=HPC),
                    )

                def out_unit(qc_, mi):
                    pj = psP.tile([P, 512], F32, tag="pj", name="pj")
                    for pp in range(NPAIR):
                        nc.tensor.matmul(
                            pj[:],
                            wo_sb[:, pp, mi * P:(mi + 1) * P],
                            Z[:, pp, qc_ * 512:(qc_ + 1) * 512],
                            start=(pp == 0),
                            stop=(pp == NPAIR - 1),
                        )
                    ot = otp.tile([P, 512], BF16, tag="ot", name="ot")
                    nc.vector.tensor_scalar_add(ot[:], pj[:], cb_sb[:, mi:mi + 1])
                    eng = nc.scalar if (qc_ * MC + mi) % 3 else nc.sync
                    eng.dma_start(
                        out[mi * P:(mi + 1) * P, qc_ * 512:(qc_ + 1) * 512], ot[:]
                    )

                filler: deque = deque()

                def pump(n=1):
                    for _ in range(n):
                        if filler:
                            filler.popleft()()

                # ---- attention ----
                def attention(p_, qc_):
                    qt, kt = qkt[p_]
                    nkc = 4 * qc_ + 4
                    zz = psZ.tile([DH + 1, 2, 512], F32, tag="zz", name="zz")
                    pend_z = deque()  # z emissions lagged one group
                    for kc in range(nkc):
                        c0 = max(0, (kc - 4 * qc_) * P)
                        diag = kc >= 4 * qc_
                        sg = psS.tile([P, 2, 512], F32, tag="sg", name="sg")
                        for nl in range(2):
                            hb = nl * DH
                            nc.tensor.matmul(
                                sg[:, nl, c0:],
                                kt[hb:hb + DH, kc * P:(kc + 1) * P],
                                qt[hb:hb + DH, qc_ * 512 + c0:(qc_ + 1) * 512],
                                start=True,
                                stop=not diag,
                            )
                        if diag:
                            for nl in range(2):
                                nc.tensor.matmul(
                                    sg[:, nl, c0:c0 + P],
                                    ident,
                                    negtri,
                                    start=False,
                                    stop=True,
                                )
                        es = esp.tile([P, 2, 512], BF16, tag="es", name="es")
                        nc.scalar.activation(
                            es[:, :, c0:],
                            sg[:, :, c0:],
                            mybir.ActivationFunctionType.Exp,
                            scale=0.125,
                        )

                        def z_emit(kc=kc, c0=c0, es=es):
                            for nl in range(2):
                                n = 2 * p_ + nl
                                nc.tensor.matmul(
                                    zz[:, nl, c0:],
                                    V_all[:, kc, n, :],
                                    es[:, nl, c0:],
                                    start=(kc == 0),
                                    stop=(kc == nkc - 1),
                                )
                        pend_z.append(z_emit)
                        if kc == 0:
                            pump(1)  # fill PE while z(kc0) waits on prev norm
                        if kc >= 1:
                            pend_z.popleft()()
                        if kc >= 2 and kc % 2 == 0:
                            pump(1)
                    while pend_z:
                        pend_z.popleft()()

                    # softmax normalize + write into Z stack
                    for nl in range(2):
                        hb = nl * DH
                        rc = rcp.tile([DH + 1, 512], BF16, tag="rc", name="rc")
                        with nc.allow_low_precision(
                            reason="bf16 rounding of softmax reciprocal"
                        ):
                            nc.vector.reciprocal(
                                rc[DH:DH + 1, :], zz[DH:DH + 1, nl, :]
                            )
                        bc = psB.tile([DH, 512], F32, tag="bc", name="bc")
                        nc.tensor.matmul(
                            bc[:],
                            ones64[DH:DH + 1, :],
                            rc[DH:DH + 1, :],
                            start=True,
                            stop=True,
                        )
                        zs = Z[hb:hb + DH, p_, qc_ * 512:(qc_ + 1) * 512]
                        nc.vector.tensor_copy(zs, zz[0:DH, nl, :])
                        nc.vector.tensor_tensor(
                            zs, zs, bc[:], mybir.AluOpType.mult
                        )
                    pump(1)

                # ---- emission schedule ----
                alloc_qk(0)
                for qc_ in range(QC):
                    for ti in (0, 1):
                        qk_unit(0, ti, qc_)
                for sc in range(4):
                    v_unit(sc)
                # p0: fill V 4..15, then qk(p1); p1: qk(p2); p2: out units
                filler.extend(
                    [lambda sc=sc: v_unit(sc) for sc in range(4, KC)]
                )
                alloc_qk(1)
                filler.extend(
                    [lambda ti=ti, qc_=qc_: qk_unit(1, ti, qc_)
                     for qc_ in range(QC) for ti in (0, 1)]
                )
                for qc_ in range(QC):
                    attention(0, qc_)
                alloc_qk(2)
                filler.extend(
                    [lambda ti=ti, qc_=qc_: qk_unit(2, ti, qc_)
                     for qc_ in range(QC) for ti in (0, 1)]
                )
                for qc_ in range(QC):
                    attention(1, qc_)
                for qc_ in range(QC):
                    attention(2, qc_)
                    filler.extend(
                        [lambda qc_=qc_, mi=mi: out_unit(qc_, mi)
                         for mi in range(MC)]
                    )
                while filler:
                    pump(1)

            if dyn_reps is not None:
                with tc.For_i(0, dyn_reps, 1):
                    emit_body()
            else:
                for _rep in range(reps):
                    emit_body()
    nc.finalize()
    return nc


def make_in_maps(inputs):
    x = np.asarray(inputs["normalized_resid_pre"], dtype=np.float32)
    W_Q = np.asarray(inputs["W_Q"], dtype=np.float32)
    W_K = np.asarray(inputs["W_K"], dtype=np.float32)
    W_V = np.asarray(inputs["W_V"], dtype=np.float32)
    W_O = np.asarray(inputs["W_O"], dtype=np.float32)
    b_Q = np.asarray(inputs["b_Q"], dtype=np.float32)
    b_K = np.asarray(inputs["b_K"], dtype=np.float32)
    b_V = np.asarray(inputs["b_V"], dtype=np.float32)
    b_O = np.asarray(inputs["b_O"], dtype=np.float32)

    bf = ml_dtypes.bfloat16
    k = np.arange(P)[:, None]
    j = np.arange(P)[None, :]
    negtri = np.where(k > j, np.float32(-240.0), np.float32(0.0))
    cst = np.concatenate([negtri, np.eye(P, dtype=np.float32)], axis=1)
    cst = np.ascontiguousarray(cst).astype(bf)

    in_maps = []
    for c in range(NCORES):
        b = c // 2
        hg = (c % 2) * HPC
        hs = slice(hg, hg + HPC)
        xT_b = np.ascontiguousarray(x[b].T).astype(bf)
        wq_c = np.ascontiguousarray(
            W_Q[hs].transpose(1, 0, 2).reshape(DM, HPC * DH)).astype(bf)
        wk_c = np.ascontiguousarray(
            W_K[hs].transpose(1, 0, 2).reshape(DM, HPC * DH)).astype(bf)
        wv_c = np.ascontiguousarray(
            W_V[hs].transpose(1, 0, 2).reshape(DM, HPC * DH)).astype(bf)
        wo_c = np.ascontiguousarray(W_O[hs].reshape(HPC * DH, DM)).astype(bf)
        bq_c = b_Q[hs].reshape(NPAIR, P).T
        bk_c = b_K[hs].reshape(NPAIR, P).T
        cvec = np.einsum("nh,nhm->m", b_V[hs], W_O[hs]).astype(np.float32)
        if hg == 0:
            cvec = cvec + b_O
        cb_c = cvec.reshape(MC, P).T
        bias_c = np.ascontiguousarray(
            np.concatenate([bq_c, bk_c, cb_c], axis=1)).astype(np.float32)
        in_maps.append({
            "xT": xT_b, "wq": wq_c, "wk": wk_c, "wv": wv_c, "wo": wo_c,
            "bias": bias_c, "cst": cst,
        })
    return in_maps


def assemble(results):
    out = np.empty((B, S, DM), dtype=np.float32)
    for b in range(B):
        acc = (np.asarray(results[2 * b]["out"], np.float32)
               + np.asarray(results[2 * b + 1]["out"], np.float32))
        out[b] = acc.T
    return out


def kernel(**inputs) -> np.ndarray:
    nc = build_nc(reps=1)
    in_maps = make_in_maps(inputs)
    res = run_bass_kernel_spmd(nc, in_maps, list(range(NCORES)))
    return assemble(res.results)



# revision 4
# speedup vs baseline: 1.0889x; 1.0889x over previous
"""Causal multi-head attention for TRN2, 8 NeuronCores — bf16 pipeline, v5.

Sharding: core c -> batch c//2, head-half c%2 (6 heads = 3 pairs).
Per-core: all matmuls bf16 (fp32 PSUM). Scores for both heads of a pair run
concurrently on disjoint PE row groups (stationary bases 0/64); diagonal
blocks get -240*upper-tri added via a matmul; one exp per k-block covers
both heads; z^T accumulates per-head into single-bank PSUM tiles (bufs=2)
so the next q-chunk's accumulation never waits on the normalize chain.
Normalize: reciprocal (DVE) + Z-copy (ACT) drain zz immediately; the
reciprocal-broadcast matmul + multiply are deferred into the filler queue
so the PE never head-of-line blocks on them. QKV/out projections are
emitted as filler units interleaved into the attention loop.

xT is double-buffered: each rep computes from slot (rep%2) while the next
rep's input DMA (split over sync+gpsimd queues) prefetches into the other
slot, so a rep never starts by waiting on input. The dynamic timing loop
runs two rep bodies per For_i iteration to halve the ~2us all-engine
back-edge barrier and let each rep tail overlap the next rep's
projections.

HW notes (measured on trn2 via A/B reps-slope): the full exp chain (ACT)
hides entirely under the PE stream; DVE PSUM-drain ops are the main
serializer; staggered_reset and hint_engines both regress this body.
"""

import numpy as np
import ml_dtypes

from concourse.bass_utils import run_bass_kernel_spmd

from collections import deque

import concourse.mybir as mybir
import concourse.tile as tile
from concourse import bacc

B, S, DM, NH, DH = 4, 2048, 768, 12, 64
P = 128
HPC = NH // 2
NPAIR = HPC // 2
MC = DM // P
QC = S // 512
KC = S // P
NCORES = 8

F32 = mybir.dt.float32
BF16 = mybir.dt.bfloat16


def build_nc(reps: int = 1, dyn_reps: int | None = None,
             staggered: bool = False, hints: bool = False,
             body_unroll: int = 2, zcopy: str = "act"):
    nc = bacc.Bacc(None, target_bir_lowering=False)

    xT = nc.dram_tensor("xT", [DM, S], BF16, kind="ExternalInput")
    wq = nc.dram_tensor("wq", [DM, HPC * DH], BF16, kind="ExternalInput")
    wk = nc.dram_tensor("wk", [DM, HPC * DH], BF16, kind="ExternalInput")
    wv = nc.dram_tensor("wv", [DM, HPC * DH], BF16, kind="ExternalInput")
    wo = nc.dram_tensor("wo", [HPC * DH, DM], BF16, kind="ExternalInput")
    bias = nc.dram_tensor("bias", [P, 2 * NPAIR + MC], F32, kind="ExternalInput")
    cst = nc.dram_tensor("cst", [P, 2 * P], BF16, kind="ExternalInput")
    out = nc.dram_tensor("out", [DM, S], BF16, kind="ExternalOutput")

    with tile.TileContext(nc) as tc:
        with (
            tc.tile_pool(name="fx", bufs=1) as fx,
            tc.tile_pool(name="qk", bufs=2) as qkp,
            tc.tile_pool(name="es", bufs=3) as esp,
            tc.tile_pool(name="rcl", bufs=3) as rcp,
            tc.tile_pool(name="ot", bufs=4) as otp,
            tc.tile_pool(name="psS", bufs=2, space="PSUM") as psS,
            tc.tile_pool(name="psZ", bufs=2, space="PSUM") as psZ,
            tc.tile_pool(name="psP", bufs=1, space="PSUM") as psP,
            tc.tile_pool(name="psB", bufs=1, space="PSUM") as psB,
        ):
            cst_sb = fx.tile([P, 2 * P], BF16, tag="cst")
            nc.gpsimd.dma_start(cst_sb[:], cst[:])
            bias_sb = fx.tile([P, 2 * NPAIR + MC], F32, tag="bias")
            nc.gpsimd.dma_start(bias_sb[:], bias[:])
            negtri = cst_sb[:, 0:P]
            ident = cst_sb[:, P:2 * P]
            bq_sb = bias_sb[:, 0:NPAIR]
            bk_sb = bias_sb[:, NPAIR:2 * NPAIR]
            cb_sb = bias_sb[:, 2 * NPAIR:]
            wq_sb = fx.tile([P, MC, HPC * DH], BF16, tag="wq")
            wk_sb = fx.tile([P, MC, HPC * DH], BF16, tag="wk")
            wv_sb = fx.tile([P, MC, HPC * DH], BF16, tag="wv")
            for w_sb, w_dr in ((wq_sb, wq), (wk_sb, wk), (wv_sb, wv)):
                nc.gpsimd.dma_start(
                    w_sb[:], w_dr.rearrange("(c p) h -> p c h", p=P)
                )
            wo_sb = fx.tile([P, NPAIR, DM], BF16, tag="wo")
            nc.gpsimd.dma_start(wo_sb[:], wo.rearrange("(c p) m -> p c m", p=P))
            ones64 = fx.tile([P, DH], BF16, tag="ones64")
            nc.gpsimd.memset(ones64[:], 1.0)
            V_all = fx.tile([P, KC, HPC, DH + 1], BF16, tag="V")
            nc.gpsimd.memset(V_all[:], 0.0)
            nc.gpsimd.memset(V_all[:, :, :, DH], 1.0)
            Z = fx.tile([P, NPAIR, S], BF16, tag="Z")
            # double-buffered xT: body with parity p computes from xts[p]
            # while prefetching the next body's input into xts[1-p]
            xts = [fx.tile([P, MC, S], BF16, tag=f"xT{i}", name=f"xT{i}")
                   for i in range(2)]

            def fill_xt(slot):
                for mi in range(MC):
                    eng = nc.sync if mi % 3 == 0 else nc.gpsimd
                    eng.dma_start(slot[:, mi, :], xT[mi * P:(mi + 1) * P, :])

            fill_xt(xts[0])

            def emit_body(parity=0):
                xT_sb = xts[parity]
                fill_xt(xts[1 - parity])

                qkt: dict[int, tuple] = {}

                def alloc_qk(p_):
                    qt = qkp.tile([P, S], BF16, tag="qt", name="qt")
                    kt = qkp.tile([P, S], BF16, tag="kt", name="kt")
                    qkt[p_] = (qt, kt)

                def qk_unit(p_, ti, qc_):
                    t_sb = qkt[p_][ti]
                    w_sb = (wq_sb, wk_sb)[ti]
                    b_sb = (bq_sb, bk_sb)[ti]
                    pj = psP.tile([P, 512], F32, tag="pj", name="pj")
                    for mi in range(MC):
                        nc.tensor.matmul(
                            pj[:],
                            w_sb[:, mi, 2 * p_ * DH:(2 * p_ + 2) * DH],
                            xT_sb[:, mi, qc_ * 512:(qc_ + 1) * 512],
                            start=(mi == 0),
                            stop=(mi == MC - 1),
                        )
                    nc.vector.tensor_scalar_add(
                        t_sb[:, qc_ * 512:(qc_ + 1) * 512], pj[:], b_sb[:, p_:p_ + 1]
                    )

                def v_unit(sc):
                    pj = psP.tile([P, 512], F32, tag="pj", name="pj")
                    vpj = pj[:, :HPC * DH]
                    for mi in range(MC):
                        nc.tensor.matmul(
                            vpj,
                            xT_sb[:, mi, sc * P:(sc + 1) * P],
                            wv_sb[:, mi, :],
                            start=(mi == 0),
                            stop=(mi == MC - 1),
                        )
                    nc.vector.tensor_copy(
                        V_all[:, sc, :, 0:DH],
                        vpj.rearrange("p (n h) -> p n h", n=HPC),
                    )

                def out_unit(qc_, mi):
                    pj = psP.tile([P, 512], F32, tag="pj", name="pj")
                    for pp in range(NPAIR):
                        nc.tensor.matmul(
                            pj[:],
                            wo_sb[:, pp, mi * P:(mi + 1) * P],
                            Z[:, pp, qc_ * 512:(qc_ + 1) * 512],
                            start=(pp == 0),
                            stop=(pp == NPAIR - 1),
                        )
                    ot = otp.tile([P, 512], BF16, tag="ot", name="ot")
                    nc.vector.tensor_scalar_add(ot[:], pj[:], cb_sb[:, mi:mi + 1])
                    eng = nc.scalar if (qc_ * MC + mi) % 3 else nc.sync
                    eng.dma_start(
                        out[mi * P:(mi + 1) * P, qc_ * 512:(qc_ + 1) * 512], ot[:]
                    )

                filler: deque = deque()

                def pump(n=1):
                    for _ in range(n):
                        if filler:
                            filler.popleft()()

                def attention(p_, qc_):
                    qt, kt = qkt[p_]
                    nkc = 4 * qc_ + 4
                    zzs = [psZ.tile([DH + 1, 512], F32, tag="zz", name="zz")
                           for _ in range(2)]
                    pend_z = deque()  # z emissions lagged one group
                    for kc in range(nkc):
                        c0 = max(0, (kc - 4 * qc_) * P)
                        diag = kc >= 4 * qc_
                        sg = psS.tile([P, 2, 512], F32, tag="sg", name="sg")
                        for nl in range(2):
                            hb = nl * DH
                            nc.tensor.matmul(
                                sg[:, nl, c0:],
                                kt[hb:hb + DH, kc * P:(kc + 1) * P],
                                qt[hb:hb + DH, qc_ * 512 + c0:(qc_ + 1) * 512],
                                start=True,
                                stop=not diag,
                            )
                        if diag:
                            for nl in range(2):
                                nc.tensor.matmul(
                                    sg[:, nl, c0:c0 + P],
                                    ident,
                                    negtri,
                                    start=False,
                                    stop=True,
                                )
                        es = esp.tile([P, 2, 512], BF16, tag="es", name="es")
                        nc.scalar.activation(
                            es[:, :, c0:],
                            sg[:, :, c0:],
                            mybir.ActivationFunctionType.Exp,
                            scale=0.125,
                        )

                        def z_emit(kc=kc, c0=c0, es=es):
                            for nl in range(2):
                                n = 2 * p_ + nl
                                nc.tensor.matmul(
                                    zzs[nl][:, c0:],
                                    V_all[:, kc, n, :],
                                    es[:, nl, c0:],
                                    start=(kc == 0),
                                    stop=(kc == nkc - 1),
                                )
                        pend_z.append(z_emit)
                        if kc == 0 or kc % 2 == 1:
                            pump(1)
                        if kc >= 1:
                            pend_z.popleft()()
                    while pend_z:
                        pend_z.popleft()()

                    # normalize: recip (DVE) + Z-copy (ACT or DVE) drain zz
                    # early; bc matmul + multiply deferred so the PE never
                    # waits on the reciprocal
                    tails = []
                    for nl in range(2):
                        hb = nl * DH
                        rc = rcp.tile([DH + 1, 512], BF16, tag="rc", name="rc")
                        zs = Z[hb:hb + DH, p_, qc_ * 512:(qc_ + 1) * 512]
                        with nc.allow_low_precision(
                            reason="bf16 rounding of softmax reciprocal"
                        ):
                            nc.vector.reciprocal(
                                rc[DH:DH + 1, :], zzs[nl][DH:DH + 1, :]
                            )
                        if zcopy == "act":
                            nc.scalar.copy(zs, zzs[nl][0:DH, :])
                        else:
                            nc.vector.tensor_copy(zs, zzs[nl][0:DH, :])

                        def norm_tail(nl=nl, hb=hb, zs=zs, rc=rc):
                            bc = psB.tile([DH, 512], F32, tag="bc", name="bc")
                            nc.tensor.matmul(
                                bc[:],
                                ones64[DH:DH + 1, :],
                                rc[DH:DH + 1, :],
                                start=True,
                                stop=True,
                            )
                            nc.vector.tensor_tensor(
                                zs, zs, bc[:], mybir.AluOpType.mult
                            )
                        tails.append(norm_tail)
                    filler.appendleft(tails[1])
                    filler.appendleft(tails[0])

                # ---- emission schedule ----
                alloc_qk(0)
                for qc_ in (0, 1):
                    for ti in (0, 1):
                        qk_unit(0, ti, qc_)
                for sc in range(4):
                    v_unit(sc)
                filler.extend(
                    [lambda ti=ti, qc_=qc_: qk_unit(0, ti, qc_)
                     for qc_ in (2, 3) for ti in (0, 1)]
                )
                filler.extend(
                    [lambda sc=sc: v_unit(sc) for sc in range(4, KC)]
                )
                alloc_qk(1)
                filler.extend(
                    [lambda ti=ti, qc_=qc_: qk_unit(1, ti, qc_)
                     for qc_ in range(QC) for ti in (0, 1)]
                )
                for qc_ in range(QC):
                    attention(0, qc_)
                alloc_qk(2)
                filler.extend(
                    [lambda ti=ti, qc_=qc_: qk_unit(2, ti, qc_)
                     for qc_ in range(QC) for ti in (0, 1)]
                )
                for qc_ in range(QC):
                    attention(1, qc_)
                for qc_ in range(QC):
                    attention(2, qc_)
                    filler.extend(
                        [lambda qc_=qc_, mi=mi: out_unit(qc_, mi)
                         for mi in range(MC)]
                    )
                while filler:
                    pump(1)

            if dyn_reps is not None:
                loop_kw = {}
                if staggered:
                    loop_kw["staggered_reset"] = True
                if hints:
                    loop_kw["hint_engines"] = tuple(mybir.ALL_ENGINES)
                assert dyn_reps % body_unroll == 0
                assert body_unroll % 2 == 0, "xT parity needs even unroll"
                with tc.For_i(0, dyn_reps // body_unroll, 1, **loop_kw):
                    for _u in range(body_unroll):
                        emit_body(_u % 2)
            else:
                for _rep in range(reps):
                    emit_body(_rep % 2)
    nc.finalize()
    return nc


def make_in_maps(inputs):
    x = np.asarray(inputs["normalized_resid_pre"], dtype=np.float32)
    W_Q = np.asarray(inputs["W_Q"], dtype=np.float32)
    W_K = np.asarray(inputs["W_K"], dtype=np.float32)
    W_V = np.asarray(inputs["W_V"], dtype=np.float32)
    W_O = np.asarray(inputs["W_O"], dtype=np.float32)
    b_Q = np.asarray(inputs["b_Q"], dtype=np.float32)
    b_K = np.asarray(inputs["b_K"], dtype=np.float32)
    b_V = np.asarray(inputs["b_V"], dtype=np.float32)
    b_O = np.asarray(inputs["b_O"], dtype=np.float32)

    bf = ml_dtypes.bfloat16
    k = np.arange(P)[:, None]
    j = np.arange(P)[None, :]
    negtri = np.where(k > j, np.float32(-240.0), np.float32(0.0))
    cst = np.concatenate([negtri, np.eye(P, dtype=np.float32)], axis=1)
    cst = np.ascontiguousarray(cst).astype(bf)

    in_maps = []
    for c in range(NCORES):
        b = c // 2
        hg = (c % 2) * HPC
        hs = slice(hg, hg + HPC)
        xT_b = np.ascontiguousarray(x[b].T).astype(bf)
        wq_c = np.ascontiguousarray(
            W_Q[hs].transpose(1, 0, 2).reshape(DM, HPC * DH)).astype(bf)
        wk_c = np.ascontiguousarray(
            W_K[hs].transpose(1, 0, 2).reshape(DM, HPC * DH)).astype(bf)
        wv_c = np.ascontiguousarray(
            W_V[hs].transpose(1, 0, 2).reshape(DM, HPC * DH)).astype(bf)
        wo_c = np.ascontiguousarray(W_O[hs].reshape(HPC * DH, DM)).astype(bf)
        bq_c = b_Q[hs].reshape(NPAIR, P).T
        bk_c = b_K[hs].reshape(NPAIR, P).T
        cvec = np.einsum("nh,nhm->m", b_V[hs], W_O[hs]).astype(np.float32)
        if hg == 0:
            cvec = cvec + b_O
        cb_c = cvec.reshape(MC, P).T
        bias_c = np.ascontiguousarray(
            np.concatenate([bq_c, bk_c, cb_c], axis=1)).astype(np.float32)
        in_maps.append({
            "xT": xT_b, "wq": wq_c, "wk": wk_c, "wv": wv_c, "wo": wo_c,
            "bias": bias_c, "cst": cst,
        })
    return in_maps


def assemble(results):
    out = np.empty((B, S, DM), dtype=np.float32)
    for b in range(B):
        acc = (np.asarray(results[2 * b]["out"], np.float32)
               + np.asarray(results[2 * b + 1]["out"], np.float32))
        out[b] = acc.T
    return out


def kernel(**inputs) -> np.ndarray:
    nc = build_nc(reps=1)
    in_maps = make_in_maps(inputs)
    res = run_bass_kernel_spmd(nc, in_maps, list(range(NCORES)))
    return assemble(res.results)

